# revision 29
# baseline (speedup 1.0000x reference)
"""Trainium2 Bass kernel for nn_Microscope (scatter_memory).

Data-parallel over batch: core c owns slab b=c (H=128, W=128, D=64).
V2 pipeline (f16 data path, f16 matmul, PE-transpose bucketing):
  1. Extraction: locs(f16)*(iota+1); per-(row, 512-chunk) top-8 values.
  2. Slot math -> flat idx; PE transpose -> per-bucket [16,128] layouts;
     sparse_gather compaction (8 w-buckets, 24 slots x 16 partitions each).
  3. Value fetch: dma_gather of 128-f16 pair rows (xy and zi interleaved
     tensors); per-(batch,tensor) tensor_tensor_reduce one-hot extraction.
  4. Placed Gaussian profiles: T = io - posoff (f32->f16), Q = T*T (2x),
     exp(-Q/4.5) on Act.  Normalization via exact 9-window sums CX/CY/CZ.
  5. rhs = (LY*amp) (x) LZ in f16 (split DVE/Pool); psum += LH.T @ rhs.
  6. Crop: Act copies fresh 16-col region, DVE adds 8-col halo; w-slab
     DMA flush as soon as final.
"""
import sys
for _p in ('/opt/trn_rl_repo',):
    if _p not in sys.path:
        sys.path.insert(0, _p)
import math
import numpy as np

import concourse.bass as bass
import concourse.bacc as bacc
import concourse.mybir as mybir
import concourse.tile as tile

F32 = mybir.dt.float32
F16 = mybir.dt.float16
I32 = mybir.dt.int32
U32 = mybir.dt.uint32
I16 = mybir.dt.int16
AF = mybir.ActivationFunctionType
OP = mybir.AluOpType
X = mybir.AxisListType.X

H, W, D = 128, 128, 64
PSF, R_ = 9, 4
INV_S2 = 1.0 / 4.5          # 1/(2*sigma^2), sigma=1.5
NBUCK, NBPB = 8, 3
NB = NBUCK * NBPB           # 24 slot-batches of 128
WJ, WK, WH = 24, 64, 128
NQ = 4                      # locs DMA quarters
IN_NAMES = ["locs", "vxy", "vzi", "scb"]
N_CORES = 8
RHS_POOL_B = frozenset((1, 3, 5, 8, 10, 13, 15, 17, 19, 21))
import os as _os
USE_TTR = _os.environ.get("K_TTR", "0") == "1"
F16_EXTRACT = _os.environ.get("K_F16X", "1") == "1"
USE_TRANSPOSE = _os.environ.get("K_TP", "1") == "1"
ACT_COPY = _os.environ.get("K_ACT", "1") == "1"
INCR_FLUSH = _os.environ.get("K_FLUSH", "1") == "1"
F16_GATHER = _os.environ.get("K_F16G", "1") == "1"


def extract_slots(nc, tc, pool, dpool, locs_d):
    """locs (f16 [128,8192]) -> per-bucket compacted flat indices.

    Returns dict: LTb [128,NB] f32 batch-layout flat idx, VT [128,NB] f32
    validity, IDX [128,192] i16 gather rows (wrapped layout)."""
    # iota value = (col % 512) + 1, valid in f16 (<= 512)
    XDT = F16 if F16_EXTRACT else F32
    io512 = pool.tile([128, 512], XDT, name="io512", tag="io512")
    nc.gpsimd.iota(io512[:], pattern=[[1, 512]], base=1, channel_multiplier=0,
                   allow_small_or_imprecise_dtypes=True)

    MX = pool.tile([128, 128], XDT, name="MX", tag="MX")
    QC = 8192 // NQ          # columns per quarter
    CPQ = QC // 512          # chunks per quarter
    for q in range(NQ):
        lq = pool.tile([128, QC], XDT, name=f"locs_q{q}", tag=f"locs_q{q}")
        if F16_EXTRACT:
            nc.sync.dma_start(out=lq[:], in_=locs_d[:, q * QC:(q + 1) * QC])
        else:
            lq16 = pool.tile([128, QC], F16, name=f"locs_h{q}",
                             tag=f"locs_h{q}")
            nc.sync.dma_start(out=lq16[:], in_=locs_d[:, q * QC:(q + 1) * QC])
            nc.vector.tensor_copy(lq[:], lq16[:])
        io_b = io512[:].rearrange("p x -> p () x").broadcast_to(
            [128, CPQ, 512])
        nc.vector.tensor_tensor(
            out=lq[:].rearrange("p (c x) -> p c x", x=512),
            in0=lq[:].rearrange("p (c x) -> p c x", x=512),
            in1=io_b, op=OP.mult)
        for cc in range(CPQ):
            c = q * CPQ + cc
            nc.vector.max(MX[:, c * 8:(c + 1) * 8],
                          lq[:, cc * 512:(cc + 1) * 512])

    # slot math: flat = (val-1) + chunk*512 + h*8192, or -1 if empty
    vld0 = pool.tile([128, 128], F32, name="vld0", tag="vld0")
    nc.vector.tensor_scalar(out=vld0[:], in0=MX[:], scalar1=0.5, scalar2=None,
                            op0=OP.is_gt)
    basei = pool.tile([128, 128], F32, name="basei", tag="basei")
    nc.gpsimd.iota(basei[:], pattern=[[512, 16], [0, 8]], base=-1,
                   channel_multiplier=8192,
                   allow_small_or_imprecise_dtypes=True)
    MX2 = pool.tile([128, 128], F32, name="MX2", tag="MX2")
    nc.vector.tensor_copy(MX2[:], MX[:])
    nc.vector.tensor_tensor(out=MX2[:], in0=MX2[:], in1=basei[:], op=OP.add)
    nc.vector.tensor_tensor(out=MX2[:], in0=MX2[:], in1=vld0[:], op=OP.mult)
    nc.vector.scalar_tensor_tensor(out=MX2[:], in0=vld0[:], scalar=1.0,
                                   in1=MX2[:], op0=OP.subtract, op1=OP.add)

    if USE_TRANSPOSE:
        # per-bucket PE transpose: SG_IN[:, t*128+h] = MX2[h, 16t+p2]
        iden = pool.tile([128, 128], F32, name="iden", tag="iden")
        nc.gpsimd.iota(iden[:], pattern=[[1, 128]], base=0,
                       channel_multiplier=-1,
                       allow_small_or_imprecise_dtypes=True)
        nc.vector.tensor_scalar(out=iden[:], in0=iden[:], scalar1=0.0,
                                scalar2=None, op0=OP.is_equal)
        SG_IN = pool.tile([16, 1024], F32, name="SG_IN", tag="SG_IN")
        with tc.tile_pool(name="tps", bufs=2, space="PSUM") as tps:
            for t in range(NBUCK):
                tp = tps.tile([16, 128], F32, name=f"tp{t}", tag="tp")
                nc.tensor.transpose(tp[:], MX2[:, 16 * t:16 * (t + 1)],
                                    iden[:])
                nc.scalar.activation(SG_IN[:, t * 128:(t + 1) * 128], tp[:],
                                     AF.Copy)
    else:
        # v0-style DRAM bounce: (128h,128c) -> SG_IN [16, 8 buckets * 128]
        flat1 = dpool.tile([1, 16384], F32, name="flat1", tag="flat1")
        f1w = flat1[:].rearrange("a (p c) -> (a p) c", p=128)
        nc.sync.dma_start(out=f1w, in_=MX2[:])
        f1r = flat1[:].rearrange("a (h16 p2 t c) -> (a p2) t c h16",
                                 h16=8, p2=16, t=8, c=16)
        SG_IN = pool.tile([16, 1024], F32, name="SG_IN", tag="SG_IN")
        sgin_v = SG_IN[:].rearrange("p (t c h16) -> p t c h16",
                                    t=8, c=16, h16=8)
        nc.sync.dma_start(out=sgin_v, in_=f1r)
    SG_OUT = pool.tile([16, 192], F32, name="SG_OUT", tag="SG_OUT")
    NF = pool.tile([1, 8], U32, name="NF", tag="NF")
    for t in range(NBUCK):
        nc.gpsimd.sparse_gather(SG_OUT[:, t * 24:(t + 1) * 24],
                                SG_IN[:, t * 128:(t + 1) * 128],
                                num_found=NF[:, t:t + 1])

    # gather row index = flat >> 6 (h*128 + w), wrapped i16 [16,192]
    LI = pool.tile([16, 192], I32, name="LI", tag="LI")
    nc.vector.tensor_copy(LI[:], SG_OUT[:])
    HWI = pool.tile([16, 192], I32, name="HWI", tag="HWI")
    nc.vector.tensor_scalar(out=HWI[:], in0=LI[:], scalar1=6, scalar2=16383,
                            op0=OP.arith_shift_right, op1=OP.bitwise_and)
    IDX16s = pool.tile([16, 192], I16, name="IDX16s", tag="IDX16s")
    nc.vector.tensor_copy(IDX16s[:], HWI[:])
    IDX = pool.tile([128, 192], I16, name="IDX", tag="IDX")
    for g in range(8):
        eng = nc.scalar if g >= 4 else nc.sync
        eng.dma_start(out=IDX[g * 16:(g + 1) * 16, :], in_=IDX16s[:])

    # bounce wrapped -> batch layout: LTb[fl*16+p2, b] = SG_OUT[p2, b*8+fl]
    flatd = dpool.tile([1, 3072], F32, name="flatd", tag="flatd")
    fw = flatd[:].rearrange("a (c p2) -> (a p2) c", p2=16)
    nc.sync.dma_start(out=fw, in_=SG_OUT[:])
    fr = flatd[:].rearrange("a (b fl p2) -> (a fl p2) b", b=NB, fl=8, p2=16)
    LTb = pool.tile([128, NB], F32, name="LTb", tag="LTb")
    nc.sync.dma_start(out=LTb[:], in_=fr)

    # batch-layout validity: VT[p, t*3+jb] = (p + 128*jb < NF[t])
    NF128 = pool.tile([128, 8], U32, name="NF128", tag="NF128")
    nc.gpsimd.partition_broadcast(NF128[:], NF[:], channels=128)
    NF128f = pool.tile([128, 8], F32, name="NF128f", tag="NF128f")
    nc.vector.tensor_copy(NF128f[:], NF128[:])
    S = pool.tile([128, NB], F32, name="S", tag="S")
    nc.gpsimd.iota(S[:], pattern=[[0, 8], [128, 3]], base=0,
                   channel_multiplier=1, allow_small_or_imprecise_dtypes=True)
    VT = pool.tile([128, NB], F32, name="VT", tag="VT")
    nf128_b = NF128f[:].rearrange("p t -> p t ()").broadcast_to([128, 8, 3])
    nc.vector.tensor_tensor(out=VT[:].rearrange("p (t j) -> p t j", j=3),
                            in0=S[:].rearrange("p (t j) -> p t j", j=3),
                            in1=nf128_b, op=OP.is_lt)
    return dict(LTb=LTb, VT=VT, IDX=IDX)


def fetch_values(nc, pool, vxy_d, vzi_d, scb_d, ex):
    """dma_gather pair rows + one-hot extraction -> vx,vy,vz,vi [128,NB] f32
    (vi scaled by scb and masked by VT) + decoded ph,pw,pd."""
    IDX, VT, LTb = ex["IDX"], ex["VT"], ex["LTb"]
    # decode positions
    LTi = pool.tile([128, NB], I32, name="LTi", tag="LTi")
    nc.vector.tensor_copy(LTi[:], LTb[:])
    phi = pool.tile([128, NB], I32, name="phi", tag="phi")
    nc.vector.tensor_scalar(out=phi[:], in0=LTi[:], scalar1=13, scalar2=None,
                            op0=OP.arith_shift_right)
    pwi = pool.tile([128, NB], I32, name="pwi", tag="pwi")
    nc.vector.tensor_scalar(out=pwi[:], in0=LTi[:], scalar1=6, scalar2=127,
                            op0=OP.arith_shift_right, op1=OP.bitwise_and)
    pdi = pool.tile([128, NB], I32, name="pdi", tag="pdi")
    nc.vector.tensor_scalar(out=pdi[:], in0=LTi[:], scalar1=63, scalar2=None,
                            op0=OP.bitwise_and)
    ph = pool.tile([128, NB], F32, name="ph_t", tag="ph_t")
    nc.vector.tensor_copy(ph[:], phi[:])
    pw = pool.tile([128, NB], F32, name="pw_t", tag="pw_t")
    nc.vector.tensor_copy(pw[:], pwi[:])
    pd = pool.tile([128, NB], F32, name="pd_t", tag="pd_t")
    nc.vector.tensor_copy(pd[:], pdi[:])

    # one-hot over d (f16)
    iod = pool.tile([128, 64], F16, name="iod", tag="iod")
    nc.gpsimd.iota(iod[:], pattern=[[1, 64]], base=0, channel_multiplier=0,
                   allow_small_or_imprecise_dtypes=True)
    OH = pool.tile([128, NB * 64], F16, name="OH", tag="OH")
    iod_b = iod[:].rearrange("p k -> p () k").broadcast_to([128, NB, 64])
    pd_b = pd[:].rearrange("p b -> p b ()").broadcast_to([128, NB, 64])
    nc.vector.tensor_tensor(out=OH[:].rearrange("p (b k) -> p b k", k=64),
                            in0=iod_b, in1=pd_b, op=OP.is_equal)

    vals = {}
    for qn, (nm, src) in enumerate((("xy", vxy_d), ("zi", vzi_d))):
        GV = pool.tile([128, NB * 128], F16, name=f"GV{nm}", tag=f"GV{nm}")
        for cg in range(3):
            nc.gpsimd.dma_gather(
                out_ap=GV[:, cg * 1024:(cg + 1) * 1024].rearrange(
                    "p (g k) -> p g k", k=128),
                in_ap=src[:].rearrange("p (r k) -> (p r) k", k=128),
                idxs_ap=IDX[:, cg * 64:(cg + 1) * 64],
                num_idxs=1024, num_idxs_reg=1024, elem_size=128,
                queue_num=qn)
        V = pool.tile([128, NB * 2], F32, name=f"V{nm}", tag=f"V{nm}")
        if USE_TTR:
            scr = pool.tile([128, 64], F16, name=f"scr{nm}", tag=f"scr{nm}")
            for b in range(NB):
                for t in range(2):
                    nc.vector.tensor_tensor_reduce(
                        out=scr[:],
                        in0=GV[:, b * 128 + t * 64:b * 128 + t * 64 + 64],
                        in1=OH[:, b * 64:(b + 1) * 64], scale=1.0, scalar=0.0,
                        op0=OP.mult, op1=OP.add,
                        accum_out=V[:, 2 * b + t:2 * b + t + 1])
        else:
            M = pool.tile([128, NB * 128], F16, name=f"M{nm}", tag=f"M{nm}")
            oh_b = OH[:].rearrange("p (b k) -> p b () k", k=64).broadcast_to(
                [128, NB, 2, 64])
            nc.vector.tensor_tensor(
                out=M[:].rearrange("p (b t k) -> p b t k", t=2, k=64),
                in0=GV[:].rearrange("p (b t k) -> p b t k", t=2, k=64),
                in1=oh_b, op=OP.mult)
            M4 = M[:].rearrange("p (bt k) -> p bt k", k=64)
            r1 = pool.tile([128, NB * 2 * 32], F16, name=f"r1{nm}",
                           tag=f"r1{nm}")
            r14 = r1[:].rearrange("p (bt k) -> p bt k", k=32)
            nc.vector.tensor_tensor(out=r14, in0=M4[:, :, 0:32],
                                    in1=M4[:, :, 32:64], op=OP.add)
            r2 = pool.tile([128, NB * 2 * 16], F16, name=f"r2{nm}",
                           tag=f"r2{nm}")
            r24 = r2[:].rearrange("p (bt k) -> p bt k", k=16)
            nc.vector.tensor_tensor(out=r24, in0=r14[:, :, 0:16],
                                    in1=r14[:, :, 16:32], op=OP.add)
            nc.vector.tensor_reduce(out=V[:], axis=X, in_=r24, op=OP.add)
        vals[nm] = V

    vx = vals["xy"][:, 0::2]
    vy = vals["xy"][:, 1::2]
    vz = vals["zi"][:, 0::2]
    vi = pool.tile([128, NB], F32, name="vi", tag="vi")
    scb_t = pool.tile([128, 1], F32, name="scb_t", tag="scb_t")
    nc.sync.dma_start(out=scb_t[:], in_=scb_d[:])
    nc.vector.tensor_scalar(out=vi[:], in0=vals["zi"][:, 1::2],
                            scalar1=scb_t[:], scalar2=None, op0=OP.mult)
    nc.vector.tensor_tensor(out=vi[:], in0=vi[:], in1=VT[:], op=OP.mult)
    return dict(ph=ph, pw=pw, pd=pd, vx=vx, vy=vy, vz=vz, vi=vi)


def eval_profile(nc, pool, posoff, Wn, io, name, buckets=None,
                 neg_posoff=None):
    """Placed unmasked Gaussian: out[p,(b,x)] = exp(-(x-posoff[p,b])^2/4.5),
    f16.  io: [128, >=Wn] f32 iota tile.  buckets=n issues the chain in n
    column groups so downstream per-bucket consumers start early.
    neg_posoff: if given, the T subtract runs per-batch on the Act engine
    (Identity with bias = -posoff)."""
    T = pool.tile([128, NB * Wn], F16, name=f"T_{name}", tag=f"T_{name}")
    Q = pool.tile([128, NB * Wn], F16, name=f"Q_{name}", tag=f"Q_{name}")
    L = pool.tile([128, NB * Wn], F16, name=f"L_{name}", tag=f"L_{name}")
    ng = buckets or 1
    bpg = NB // ng
    for g in range(ng):
        bsl = slice(g * bpg, (g + 1) * bpg)
        csl = slice(g * bpg * Wn, (g + 1) * bpg * Wn)
        T3 = T[:, csl].rearrange("p (b x) -> p b x", x=Wn)
        if neg_posoff is not None:
            for bb in range(g * bpg, (g + 1) * bpg):
                nc.scalar.activation(
                    T[:, bb * Wn:(bb + 1) * Wn], io[:, :Wn], AF.Identity,
                    bias=neg_posoff[:, bb:bb + 1])
        else:
            io_b = io[:, :Wn].rearrange("p x -> p () x").broadcast_to(
                [128, bpg, Wn])
            po_b = posoff[:, bsl].rearrange("p b -> p b ()").broadcast_to(
                [128, bpg, Wn])
            nc.vector.tensor_tensor(out=T3, in0=io_b, in1=po_b,
                                    op=OP.subtract)
        nc.vector.tensor_tensor(out=Q[:, csl], in0=T[:, csl], in1=T[:, csl],
                                op=OP.mult)
        nc.scalar.activation(L[:, csl], Q[:, csl], AF.Exp, scale=-INV_S2)
    return L


def body(tc, outs, ins):
    nc = tc.nc
    out_d = outs[0]
    locs_d, vxy_d, vzi_d, scb_d = ins
    with (tc.tile_pool(name="pool", bufs=1) as pool,
          tc.tile_pool(name="rhsp", bufs=6) as rhsp,
          tc.tile_pool(name="psump", bufs=2, space="PSUM") as psump,
          tc.tile_pool(name="dram", bufs=1, space="DRAM") as dpool):
        warm = pool.tile([128, 1], F32, name="warm", tag="warm")
        nc.gpsimd.memset(warm[:], 0.0)
        nc.scalar.activation(warm[:], warm[:], AF.Exp, scale=-1.0)
        ex = extract_slots(nc, tc, pool, dpool, locs_d)
        tb_ = fetch_values(nc, pool, vxy_d, vzi_d, scb_d, ex)
        ph, pw, pd = tb_["ph"], tb_["pw"], tb_["pd"]
        vx, vy, vz, vi = tb_["vx"], tb_["vy"], tb_["vz"], tb_["vi"]

        # position offsets
        io128 = pool.tile([128, 128], F32, name="io128", tag="io128")
        nc.gpsimd.iota(io128[:], pattern=[[1, 128]], base=0,
                       channel_multiplier=0,
                       allow_small_or_imprecise_dtypes=True)
        woff = pool.tile([128, NB], F32, name="woff", tag="woff")
        nc.gpsimd.iota(woff[:], pattern=[[16, NBUCK], [0, NBPB]], base=-4,
                       channel_multiplier=0,
                       allow_small_or_imprecise_dtypes=True)
        pox = pool.tile([128, NB], F32, name="pox", tag="pox")
        nc.vector.tensor_tensor(out=pox[:], in0=ph[:], in1=vx, op=OP.add)
        poy = pool.tile([128, NB], F32, name="poy", tag="poy")
        nc.vector.tensor_tensor(out=poy[:], in0=pw[:], in1=woff[:],
                                op=OP.subtract)
        nc.vector.tensor_tensor(out=poy[:], in0=poy[:], in1=vy, op=OP.add)
        poz = pool.tile([128, NB], F32, name="poz", tag="poz")
        nc.vector.tensor_tensor(out=poz[:], in0=pd[:], in1=vz, op=OP.add)
        # 9-grid centers: 4 + offset
        pcx = pool.tile([128, NB], F32, name="pcx", tag="pcx")
        nc.vector.tensor_scalar(out=pcx[:], in0=vx, scalar1=4.0, scalar2=None,
                                op0=OP.add)
        pcy = pool.tile([128, NB], F32, name="pcy", tag="pcy")
        nc.vector.tensor_scalar(out=pcy[:], in0=vy, scalar1=4.0, scalar2=None,
                                op0=OP.add)
        pcz = pool.tile([128, NB], F32, name="pcz", tag="pcz")
        nc.vector.tensor_scalar(out=pcz[:], in0=vz, scalar1=4.0, scalar2=None,
                                op0=OP.add)

        CX = eval_profile(nc, pool, pcx, PSF, io128, "cx")
        CY = eval_profile(nc, pool, pcy, PSF, io128, "cy")
        CZ = eval_profile(nc, pool, pcz, PSF, io128, "cz")
        npox = pool.tile([128, NB], F32, name="npox", tag="npox")
        nc.vector.tensor_scalar(out=npox[:], in0=pox[:], scalar1=-1.0,
                                scalar2=None, op0=OP.mult)
        # merged per-bucket profile block: [y(3x24)=72 | z(3x64)=192 | x(3x128)=384]
        BLK = NBPB * (WJ + WK + WH)        # 648
        OY, OZ, OX = 0, NBPB * WJ, NBPB * (WJ + WK)
        T_all = pool.tile([128, NBUCK * BLK], F16, name="T_all", tag="T_all")
        Q_all = pool.tile([128, NBUCK * BLK], F16, name="Q_all", tag="Q_all")
        L_all = pool.tile([128, NBUCK * BLK], F16, name="L_all", tag="L_all")

        def ly_sl(b):
            return slice((b // 3) * BLK + OY + (b % 3) * WJ,
                         (b // 3) * BLK + OY + (b % 3) * WJ + WJ)

        def lz_sl(b):
            return slice((b // 3) * BLK + OZ + (b % 3) * WK,
                         (b // 3) * BLK + OZ + (b % 3) * WK + WK)

        def lh_sl(b):
            return slice((b // 3) * BLK + OX + (b % 3) * WH,
                         (b // 3) * BLK + OX + (b % 3) * WH + WH)

        for g in range(NBUCK):
            base = g * BLK
            bsl = slice(g * NBPB, (g + 1) * NBPB)
            for off, po, Wn in ((OY, poy, WJ), (OZ, poz, WK)):
                csl = slice(base + off, base + off + NBPB * Wn)
                io_b = io128[:, :Wn].rearrange("p x -> p () x").broadcast_to(
                    [128, NBPB, Wn])
                po_b = po[:, bsl].rearrange("p b -> p b ()").broadcast_to(
                    [128, NBPB, Wn])
                nc.vector.tensor_tensor(
                    out=T_all[:, csl].rearrange("p (b x) -> p b x", x=Wn),
                    in0=io_b, in1=po_b, op=OP.subtract)
            for bb in range(g * NBPB, (g + 1) * NBPB):
                xs = base + OX + (bb % 3) * WH
                nc.scalar.activation(
                    T_all[:, xs:xs + WH], io128[:, :WH], AF.Identity,
                    bias=npox[:, bb:bb + 1])
            blk = slice(base, base + BLK)
            nc.vector.tensor_tensor(out=Q_all[:, blk], in0=T_all[:, blk],
                                    in1=T_all[:, blk], op=OP.mult)
            nc.scalar.activation(L_all[:, blk], Q_all[:, blk], AF.Exp,
                                 scale=-INV_S2)

        # normalization: amp = vi / (sum CX * sum CY * sum CZ)
        s_x = pool.tile([128, NB], F32, name="sx", tag="sx")
        s_y = pool.tile([128, NB], F32, name="sy", tag="sy")
        s_z = pool.tile([128, NB], F32, name="sz", tag="sz")
        for s_t, C in ((s_x, CX), (s_y, CY), (s_z, CZ)):
            nc.vector.tensor_reduce(
                out=s_t[:], axis=X,
                in_=C[:].rearrange("p (b x) -> p b x", x=PSF), op=OP.add)
        nc.vector.tensor_tensor(out=s_x[:], in0=s_x[:], in1=s_y[:], op=OP.mult)
        nc.vector.tensor_tensor(out=s_x[:], in0=s_x[:], in1=s_z[:], op=OP.mult)
        nc.vector.reciprocal(s_y[:], s_x[:])
        amp = pool.tile([128, NB], F32, name="amp", tag="amp")
        nc.vector.tensor_tensor(out=amp[:], in0=s_y[:], in1=vi[:], op=OP.mult)
        LHa = pool.tile([128, NB * WH], F16, name="LHa", tag="LHa")

        out_t = pool.tile([128, W * D], F32, name="out_t", tag="out_t")
        flushed = 0
        for tb in range(NBUCK):
            ps = psump.tile([128, WJ * WK], F32, name=f"ps{tb}", tag="ps")
            for b in range(tb * NBPB, (tb + 1) * NBPB):
                nc.scalar.activation(
                    LHa[:, b * WH:(b + 1) * WH], L_all[:, lh_sl(b)],
                    AF.Identity, scale=amp[:, b:b + 1])
            for j in range(NBPB):
                b = tb * NBPB + j
                rhs = rhsp.tile([128, WJ * WK], F16, name=f"rhs{b}", tag="rhs")
                rhs3 = rhs[:].rearrange("p (j k) -> p j k", k=WK)
                ly_b = L_all[:, ly_sl(b)].rearrange(
                    "p j -> p j ()").broadcast_to([128, WJ, WK])
                lz_b = L_all[:, lz_sl(b)].rearrange(
                    "p k -> p () k").broadcast_to([128, WJ, WK])
                eng = nc.gpsimd if b in RHS_POOL_B else nc.vector
                eng.tensor_tensor(out=rhs3, in0=ly_b, in1=lz_b, op=OP.mult)
                for c0 in range(0, WJ * WK, 512):
                    c1 = min(c0 + 512, WJ * WK)
                    nc.tensor.matmul(ps[:, c0:c1], lhsT=LHa[:, bass.ts(b, WH)],
                                     rhs=rhs[:, c0:c1],
                                     start=(j == 0), stop=(j == NBPB - 1))
            # crop: fresh region copy (Act), halo region add (DVE)
            ps3 = ps[:].rearrange("p (j k) -> p j k", k=WK)
            w_new0 = 16 * tb + 4 if tb > 0 else 0
            w_new1 = min(16 * tb + 20, W)
            j_new0 = w_new0 - (16 * tb - 4)
            out_new = out_t[:, w_new0 * D:w_new1 * D].rearrange(
                "p (j k) -> p j k", k=D)
            if ACT_COPY:
                nc.scalar.activation(
                    out_new, ps3[:, j_new0:j_new0 + (w_new1 - w_new0), :],
                    AF.Copy)
            else:
                nc.vector.tensor_copy(
                    out_new, ps3[:, j_new0:j_new0 + (w_new1 - w_new0), :])
            if tb > 0:
                w_h0 = 16 * tb - 4
                out_h = out_t[:, w_h0 * D:(w_h0 + 8) * D].rearrange(
                    "p (j k) -> p j k", k=D)
                nc.vector.tensor_tensor(
                    out=out_h, in0=out_h, in1=ps3[:, 0:8, :],
                    op=OP.add)
            if INCR_FLUSH:
                # flush final columns [flushed, 16tb+12)
                f1 = 16 * tb + 12 if tb < NBUCK - 1 else W
                nc.sync.dma_start(out=out_d[:, flushed * D:f1 * D],
                                  in_=out_t[:, flushed * D:f1 * D])
                flushed = f1
        if not INCR_FLUSH:
            nc.sync.dma_start(out=out_d[:], in_=out_t[:])


def build_nc(repeats=1):
    nc = bacc.Bacc("TRN2", target_bir_lowering=False, debug=False,
                   num_devices=N_CORES, dynamic_dma_scratch_size=65536,
                   num_swdge_queues=2)
    ins = []
    for nm in IN_NAMES:
        if nm == "scb":
            shape, dt = [128, 1], F32
        elif nm == "locs":
            shape, dt = [128, 8192], F16
        else:
            shape, dt = [128, 16384], F16
        ins.append(nc.dram_tensor(nm, shape, dt, kind="ExternalInput").ap())
    out_d = nc.dram_tensor("out", [128, W * D], F32, kind="ExternalOutput").ap()
    with tile.TileContext(nc) as tc:
        for _rep in range(repeats):
            body(tc, [out_d], ins)
    nc.compile()
    return nc


class _SpmdRunner:
    def __init__(self, nc, n_cores=N_CORES):
        import jax
        import jax.numpy as jnp
        from jax.sharding import Mesh, PartitionSpec
        from jax.experimental.shard_map import shard_map
        from concourse import bass2jax
        from concourse.bass2jax import _bass_exec_p, partition_id_tensor
        bass2jax.install_neuronx_cc_hook()
        self.jax, self.jnp = jax, jnp
        self.n_cores = n_cores
        in_names, out_names, out_avals, zero_outs = [], [], [], []
        pname = nc.partition_id_tensor.name if nc.partition_id_tensor else None
        for alloc in nc.m.functions[0].allocations:
            if not isinstance(alloc, mybir.MemoryLocationSet):
                continue
            name = alloc.memorylocations[0].name
            if alloc.kind == "ExternalInput":
                if name != pname:
                    in_names.append(name)
            elif alloc.kind == "ExternalOutput":
                shape = tuple(alloc.tensor_shape)
                dtype = mybir.dt.np(alloc.dtype)
                out_names.append(name)
                out_avals.append(jax.core.ShapedArray(shape, dtype))
                zero_outs.append(np.zeros(shape, dtype))
        self.in_names, self.out_names = in_names, out_names
        self.out_avals, self.zero_outs = out_avals, zero_outs
        n_params, n_outs = len(in_names), len(out_avals)
        all_in = in_names + out_names + ([pname] if pname else [])

        def _fn(*args):
            operands = list(args)
            if pname is not None:
                operands.append(partition_id_tensor())
            return tuple(_bass_exec_p.bind(
                *operands, out_avals=tuple(out_avals), in_names=tuple(all_in),
                out_names=tuple(out_names), lowering_input_output_aliases=(),
                sim_require_finite=True, sim_require_nnan=True, nc=nc))

        devices = jax.devices()[:n_cores]
        mesh = Mesh(np.asarray(devices), ("core",))
        specs = (PartitionSpec("core"),)
        self.sharded = jax.jit(
            shard_map(_fn, mesh=mesh, in_specs=specs * (n_params + n_outs),
                      out_specs=specs * n_outs),
            donate_argnums=tuple(range(n_params, n_params + n_outs)),
            keep_unused=True)

    def run(self, in_maps):
        concat = [np.concatenate([np.asarray(m[n]) for m in in_maps], axis=0)
                  for n in self.in_names]
        zeros = [self.jnp.zeros((self.n_cores * z.shape[0], *z.shape[1:]),
                                z.dtype) for z in self.zero_outs]
        outs = self.sharded(*concat, *zeros)
        self.jax.block_until_ready(outs)
        return [
            {n: np.asarray(outs[i]).reshape(self.n_cores,
                                            *self.out_avals[i].shape)[c]
             for i, n in enumerate(self.out_names)}
            for c in range(self.n_cores)]


_RUNNER_CACHE = {}


def _get_runner(repeats=1):
    if repeats not in _RUNNER_CACHE:
        _RUNNER_CACHE[repeats] = _SpmdRunner(build_nc(repeats))
    return _RUNNER_CACHE[repeats]


def _make_in_maps(locs_3d, x_os_3d, y_os_3d, z_os_3d, ints_3d, scale):
    sc = float(np.asarray(scale).reshape(-1)[0])
    scb = np.full((128, 1), 1000.0 * sc, np.float32)
    in_maps = []
    for c in range(N_CORES):
        xs = np.asarray(x_os_3d)[c, 0]
        ys = np.asarray(y_os_3d)[c, 0]
        zs = np.asarray(z_os_3d)[c, 0]
        iv = np.asarray(ints_3d)[c, 0]
        vxy = np.stack([xs.reshape(128, 128, 64), ys.reshape(128, 128, 64)],
                       axis=2).reshape(128, 16384).astype(np.float16)
        vzi = np.stack([zs.reshape(128, 128, 64), iv.reshape(128, 128, 64)],
                       axis=2).reshape(128, 16384).astype(np.float16)
        m = {"scb": scb,
             "locs": np.asarray(locs_3d)[c, 0].reshape(128, 8192).astype(
                 np.float16),
             "vxy": np.ascontiguousarray(vxy),
             "vzi": np.ascontiguousarray(vzi)}
        in_maps.append(m)
    return in_maps


def kernel(locs_3d, x_os_3d, y_os_3d, z_os_3d, ints_3d, scale):
    runner = _get_runner()
    in_maps = _make_in_maps(locs_3d, x_os_3d, y_os_3d, z_os_3d, ints_3d, scale)
    res = runner.run(in_maps)
    out = np.stack([res[c]["out"].reshape(H, W, D) for c in range(N_CORES)])
    return out[:, None].astype(np.float32)


# revision 30
# speedup vs baseline: 1.5857x; 1.5857x over previous
"""Trainium2 Bass kernel for nn_Microscope (scatter_memory).

Data-parallel over batch: core c owns slab b=c (H=128, W=128, D=64).
V2 pipeline (f16 data path, f16 matmul, PE-transpose bucketing):
  1. Extraction: locs(f16)*(iota+1); per-(row, 512-chunk) top-8 values.
  2. Slot math -> flat idx; PE transpose -> per-bucket [16,128] layouts;
     sparse_gather compaction (8 w-buckets, 24 slots x 16 partitions each).
  3. Value fetch: dma_gather of 128-f16 pair rows (xy and zi interleaved
     tensors); per-(batch,tensor) tensor_tensor_reduce one-hot extraction.
  4. Placed Gaussian profiles: T = io - posoff (f32->f16), Q = T*T (2x),
     exp(-Q/4.5) on Act.  Normalization via exact 9-window sums CX/CY/CZ.
  5. rhs = (LY*amp) (x) LZ in f16 (split DVE/Pool); psum += LH.T @ rhs.
  6. Crop: Act copies fresh 16-col region, DVE adds 8-col halo; w-slab
     DMA flush as soon as final.
"""
import sys
for _p in ('/opt/trn_rl_repo',):
    if _p not in sys.path:
        sys.path.insert(0, _p)
import math
import numpy as np

import concourse.bass as bass
import concourse.bacc as bacc
import concourse.mybir as mybir
import concourse.tile as tile

F32 = mybir.dt.float32
F16 = mybir.dt.float16
I32 = mybir.dt.int32
U32 = mybir.dt.uint32
I16 = mybir.dt.int16
AF = mybir.ActivationFunctionType
OP = mybir.AluOpType
X = mybir.AxisListType.X

H, W, D = 128, 128, 64
PSF, R_ = 9, 4
INV_S2 = 1.0 / 4.5          # 1/(2*sigma^2), sigma=1.5
NBUCK, NBPB = 8, 3
NB = NBUCK * NBPB           # 24 slot-batches of 128
WJ, WK, WH = 24, 64, 128
NQ = 4                      # locs DMA quarters
IN_NAMES = ["locs", "vxy", "vzi", "scb"]
N_CORES = 8
RHS_POOL_B = frozenset((1, 3, 5, 8, 10, 13, 15, 17, 19, 21))
import os as _os
USE_TTR = _os.environ.get("K_TTR", "0") == "1"
F16_EXTRACT = _os.environ.get("K_F16X", "1") == "1"
USE_TRANSPOSE = _os.environ.get("K_TP", "1") == "1"
ACT_COPY = _os.environ.get("K_ACT", "1") == "1"
INCR_FLUSH = _os.environ.get("K_FLUSH", "1") == "1"
F16_GATHER = _os.environ.get("K_F16G", "1") == "1"


def extract_slots(nc, tc, pool, dpool, locs_d):
    """locs (f16 [128,8192]) -> per-bucket compacted flat indices.

    Returns dict: LTb [128,NB] f32 batch-layout flat idx, VT [128,NB] f32
    validity, IDX [128,192] i16 gather rows (wrapped layout)."""
    # iota value = (col % 512) + 1, valid in f16 (<= 512)
    XDT = F16 if F16_EXTRACT else F32
    io512 = pool.tile([128, 512], XDT, name="io512", tag="io512")
    nc.gpsimd.iota(io512[:], pattern=[[1, 512]], base=1, channel_multiplier=0,
                   allow_small_or_imprecise_dtypes=True)

    MX = pool.tile([128, 128], XDT, name="MX", tag="MX")
    QC = 8192 // NQ          # columns per quarter
    CPQ = QC // 512          # chunks per quarter
    for q in range(NQ):
        lq = pool.tile([128, QC], XDT, name=f"locs_q{q}", tag=f"locs_q{q}")
        if F16_EXTRACT:
            nc.sync.dma_start(out=lq[:], in_=locs_d[:, q * QC:(q + 1) * QC])
        else:
            lq16 = pool.tile([128, QC], F16, name=f"locs_h{q}",
                             tag=f"locs_h{q}")
            nc.sync.dma_start(out=lq16[:], in_=locs_d[:, q * QC:(q + 1) * QC])
            nc.vector.tensor_copy(lq[:], lq16[:])
        io_b = io512[:].rearrange("p x -> p () x").broadcast_to(
            [128, CPQ, 512])
        nc.vector.tensor_tensor(
            out=lq[:].rearrange("p (c x) -> p c x", x=512),
            in0=lq[:].rearrange("p (c x) -> p c x", x=512),
            in1=io_b, op=OP.mult)
        for cc in range(CPQ):
            c = q * CPQ + cc
            nc.vector.max(MX[:, c * 8:(c + 1) * 8],
                          lq[:, cc * 512:(cc + 1) * 512])

    # slot math: flat = (val-1) + chunk*512 + h*8192, or -1 if empty
    vld0 = pool.tile([128, 128], F32, name="vld0", tag="vld0")
    nc.vector.tensor_scalar(out=vld0[:], in0=MX[:], scalar1=0.5, scalar2=None,
                            op0=OP.is_gt)
    basei = pool.tile([128, 128], F32, name="basei", tag="basei")
    nc.gpsimd.iota(basei[:], pattern=[[512, 16], [0, 8]], base=-1,
                   channel_multiplier=8192,
                   allow_small_or_imprecise_dtypes=True)
    MX2 = pool.tile([128, 128], F32, name="MX2", tag="MX2")
    nc.vector.tensor_copy(MX2[:], MX[:])
    nc.vector.tensor_tensor(out=MX2[:], in0=MX2[:], in1=basei[:], op=OP.add)
    nc.vector.tensor_tensor(out=MX2[:], in0=MX2[:], in1=vld0[:], op=OP.mult)
    nc.vector.scalar_tensor_tensor(out=MX2[:], in0=vld0[:], scalar=1.0,
                                   in1=MX2[:], op0=OP.subtract, op1=OP.add)

    if USE_TRANSPOSE:
        # per-bucket PE transpose: SG_IN[:, t*128+h] = MX2[h, 16t+p2]
        iden = pool.tile([128, 128], F32, name="iden", tag="iden")
        nc.gpsimd.iota(iden[:], pattern=[[1, 128]], base=0,
                       channel_multiplier=-1,
                       allow_small_or_imprecise_dtypes=True)
        nc.vector.tensor_scalar(out=iden[:], in0=iden[:], scalar1=0.0,
                                scalar2=None, op0=OP.is_equal)
        SG_IN = pool.tile([16, 1024], F32, name="SG_IN", tag="SG_IN")
        with tc.tile_pool(name="tps", bufs=2, space="PSUM") as tps:
            for t in range(NBUCK):
                tp = tps.tile([16, 128], F32, name=f"tp{t}", tag="tp")
                nc.tensor.transpose(tp[:], MX2[:, 16 * t:16 * (t + 1)],
                                    iden[:])
                nc.scalar.activation(SG_IN[:, t * 128:(t + 1) * 128], tp[:],
                                     AF.Copy)
    else:
        # v0-style DRAM bounce: (128h,128c) -> SG_IN [16, 8 buckets * 128]
        flat1 = dpool.tile([1, 16384], F32, name="flat1", tag="flat1")
        f1w = flat1[:].rearrange("a (p c) -> (a p) c", p=128)
        nc.sync.dma_start(out=f1w, in_=MX2[:])
        f1r = flat1[:].rearrange("a (h16 p2 t c) -> (a p2) t c h16",
                                 h16=8, p2=16, t=8, c=16)
        SG_IN = pool.tile([16, 1024], F32, name="SG_IN", tag="SG_IN")
        sgin_v = SG_IN[:].rearrange("p (t c h16) -> p t c h16",
                                    t=8, c=16, h16=8)
        nc.sync.dma_start(out=sgin_v, in_=f1r)
    SG_OUT = pool.tile([16, 192], F32, name="SG_OUT", tag="SG_OUT")
    NF = pool.tile([1, 8], U32, name="NF", tag="NF")
    for t in range(NBUCK):
        nc.gpsimd.sparse_gather(SG_OUT[:, t * 24:(t + 1) * 24],
                                SG_IN[:, t * 128:(t + 1) * 128],
                                num_found=NF[:, t:t + 1])

    # gather row index = flat >> 6 (h*128 + w), wrapped i16 [16,192]
    LI = pool.tile([16, 192], I32, name="LI", tag="LI")
    nc.vector.tensor_copy(LI[:], SG_OUT[:])
    HWI = pool.tile([16, 192], I32, name="HWI", tag="HWI")
    nc.vector.tensor_scalar(out=HWI[:], in0=LI[:], scalar1=6, scalar2=16383,
                            op0=OP.arith_shift_right, op1=OP.bitwise_and)
    IDX16s = pool.tile([16, 192], I16, name="IDX16s", tag="IDX16s")
    nc.vector.tensor_copy(IDX16s[:], HWI[:])
    IDX = pool.tile([128, 192], I16, name="IDX", tag="IDX")
    for g in range(8):
        eng = nc.scalar if g >= 4 else nc.sync
        eng.dma_start(out=IDX[g * 16:(g + 1) * 16, :], in_=IDX16s[:])

    # bounce wrapped -> batch layout: LTb[fl*16+p2, b] = SG_OUT[p2, b*8+fl]
    flatd = dpool.tile([1, 3072], F32, name="flatd", tag="flatd")
    fw = flatd[:].rearrange("a (c p2) -> (a p2) c", p2=16)
    nc.sync.dma_start(out=fw, in_=SG_OUT[:])
    fr = flatd[:].rearrange("a (b fl p2) -> (a fl p2) b", b=NB, fl=8, p2=16)
    LTb = pool.tile([128, NB], F32, name="LTb", tag="LTb")
    nc.sync.dma_start(out=LTb[:], in_=fr)

    # batch-layout validity: VT[p, t*3+jb] = (p + 128*jb < NF[t])
    NF128 = pool.tile([128, 8], U32, name="NF128", tag="NF128")
    nc.gpsimd.partition_broadcast(NF128[:], NF[:], channels=128)
    NF128f = pool.tile([128, 8], F32, name="NF128f", tag="NF128f")
    nc.vector.tensor_copy(NF128f[:], NF128[:])
    S = pool.tile([128, NB], F32, name="S", tag="S")
    nc.gpsimd.iota(S[:], pattern=[[0, 8], [128, 3]], base=0,
                   channel_multiplier=1, allow_small_or_imprecise_dtypes=True)
    VT = pool.tile([128, NB], F32, name="VT", tag="VT")
    nf128_b = NF128f[:].rearrange("p t -> p t ()").broadcast_to([128, 8, 3])
    nc.vector.tensor_tensor(out=VT[:].rearrange("p (t j) -> p t j", j=3),
                            in0=S[:].rearrange("p (t j) -> p t j", j=3),
                            in1=nf128_b, op=OP.is_lt)
    return dict(LTb=LTb, VT=VT, IDX=IDX)


def fetch_values(nc, pool, vxy_d, vzi_d, scb_d, ex):
    """dma_gather pair rows + one-hot extraction -> vx,vy,vz,vi [128,NB] f32
    (vi scaled by scb and masked by VT) + decoded ph,pw,pd."""
    IDX, VT, LTb = ex["IDX"], ex["VT"], ex["LTb"]
    # decode positions
    LTi = pool.tile([128, NB], I32, name="LTi", tag="LTi")
    nc.vector.tensor_copy(LTi[:], LTb[:])
    phi = pool.tile([128, NB], I32, name="phi", tag="phi")
    nc.vector.tensor_scalar(out=phi[:], in0=LTi[:], scalar1=13, scalar2=None,
                            op0=OP.arith_shift_right)
    pwi = pool.tile([128, NB], I32, name="pwi", tag="pwi")
    nc.vector.tensor_scalar(out=pwi[:], in0=LTi[:], scalar1=6, scalar2=127,
                            op0=OP.arith_shift_right, op1=OP.bitwise_and)
    pdi = pool.tile([128, NB], I32, name="pdi", tag="pdi")
    nc.vector.tensor_scalar(out=pdi[:], in0=LTi[:], scalar1=63, scalar2=None,
                            op0=OP.bitwise_and)
    ph = pool.tile([128, NB], F32, name="ph_t", tag="ph_t")
    nc.vector.tensor_copy(ph[:], phi[:])
    pw = pool.tile([128, NB], F32, name="pw_t", tag="pw_t")
    nc.vector.tensor_copy(pw[:], pwi[:])
    pd = pool.tile([128, NB], F32, name="pd_t", tag="pd_t")
    nc.vector.tensor_copy(pd[:], pdi[:])

    # one-hot over d (f16)
    iod = pool.tile([128, 64], F16, name="iod", tag="iod")
    nc.gpsimd.iota(iod[:], pattern=[[1, 64]], base=0, channel_multiplier=0,
                   allow_small_or_imprecise_dtypes=True)
    OH = pool.tile([128, NB * 64], F16, name="OH", tag="OH")
    iod_b = iod[:].rearrange("p k -> p () k").broadcast_to([128, NB, 64])
    pd_b = pd[:].rearrange("p b -> p b ()").broadcast_to([128, NB, 64])
    nc.vector.tensor_tensor(out=OH[:].rearrange("p (b k) -> p b k", k=64),
                            in0=iod_b, in1=pd_b, op=OP.is_equal)

    vals = {}
    for qn, (nm, src) in enumerate((("xy", vxy_d), ("zi", vzi_d))):
        GV = pool.tile([128, NB * 128], F16, name=f"GV{nm}", tag=f"GV{nm}")
        for cg in range(3):
            nc.gpsimd.dma_gather(
                out_ap=GV[:, cg * 1024:(cg + 1) * 1024].rearrange(
                    "p (g k) -> p g k", k=128),
                in_ap=src[:].rearrange("p (r k) -> (p r) k", k=128),
                idxs_ap=IDX[:, cg * 64:(cg + 1) * 64],
                num_idxs=1024, num_idxs_reg=1024, elem_size=128,
                queue_num=qn)
        V = pool.tile([128, NB * 2], F32, name=f"V{nm}", tag=f"V{nm}")
        if USE_TTR:
            scr = pool.tile([128, 64], F16, name=f"scr{nm}", tag=f"scr{nm}")
            for b in range(NB):
                for t in range(2):
                    nc.vector.tensor_tensor_reduce(
                        out=scr[:],
                        in0=GV[:, b * 128 + t * 64:b * 128 + t * 64 + 64],
                        in1=OH[:, b * 64:(b + 1) * 64], scale=1.0, scalar=0.0,
                        op0=OP.mult, op1=OP.add,
                        accum_out=V[:, 2 * b + t:2 * b + t + 1])
        else:
            M = pool.tile([128, NB * 128], F16, name=f"M{nm}", tag=f"M{nm}")
            oh_b = OH[:].rearrange("p (b k) -> p b () k", k=64).broadcast_to(
                [128, NB, 2, 64])
            nc.vector.tensor_tensor(
                out=M[:].rearrange("p (b t k) -> p b t k", t=2, k=64),
                in0=GV[:].rearrange("p (b t k) -> p b t k", t=2, k=64),
                in1=oh_b, op=OP.mult)
            M4 = M[:].rearrange("p (bt k) -> p bt k", k=64)
            r1 = pool.tile([128, NB * 2 * 32], F16, name=f"r1{nm}",
                           tag=f"r1{nm}")
            r14 = r1[:].rearrange("p (bt k) -> p bt k", k=32)
            nc.vector.tensor_tensor(out=r14, in0=M4[:, :, 0:32],
                                    in1=M4[:, :, 32:64], op=OP.add)
            r2 = pool.tile([128, NB * 2 * 16], F16, name=f"r2{nm}",
                           tag=f"r2{nm}")
            r24 = r2[:].rearrange("p (bt k) -> p bt k", k=16)
            nc.vector.tensor_tensor(out=r24, in0=r14[:, :, 0:16],
                                    in1=r14[:, :, 16:32], op=OP.add)
            nc.vector.tensor_reduce(out=V[:], axis=X, in_=r24, op=OP.add)
        vals[nm] = V

    vx = vals["xy"][:, 0::2]
    vy = vals["xy"][:, 1::2]
    vz = vals["zi"][:, 0::2]
    vi = pool.tile([128, NB], F32, name="vi", tag="vi")
    scb_t = pool.tile([128, 1], F32, name="scb_t", tag="scb_t")
    nc.sync.dma_start(out=scb_t[:], in_=scb_d[:])
    nc.vector.tensor_scalar(out=vi[:], in0=vals["zi"][:, 1::2],
                            scalar1=scb_t[:], scalar2=None, op0=OP.mult)
    nc.vector.tensor_tensor(out=vi[:], in0=vi[:], in1=VT[:], op=OP.mult)
    return dict(ph=ph, pw=pw, pd=pd, vx=vx, vy=vy, vz=vz, vi=vi)


def eval_profile(nc, pool, posoff, Wn, io, name, buckets=None,
                 neg_posoff=None):
    """Placed unmasked Gaussian: out[p,(b,x)] = exp(-(x-posoff[p,b])^2/4.5),
    f16.  io: [128, >=Wn] f32 iota tile.  buckets=n issues the chain in n
    column groups so downstream per-bucket consumers start early.
    neg_posoff: if given, the T subtract runs per-batch on the Act engine
    (Identity with bias = -posoff)."""
    T = pool.tile([128, NB * Wn], F16, name=f"T_{name}", tag=f"T_{name}")
    Q = pool.tile([128, NB * Wn], F16, name=f"Q_{name}", tag=f"Q_{name}")
    L = pool.tile([128, NB * Wn], F16, name=f"L_{name}", tag=f"L_{name}")
    ng = buckets or 1
    bpg = NB // ng
    for g in range(ng):
        bsl = slice(g * bpg, (g + 1) * bpg)
        csl = slice(g * bpg * Wn, (g + 1) * bpg * Wn)
        T3 = T[:, csl].rearrange("p (b x) -> p b x", x=Wn)
        if neg_posoff is not None:
            for bb in range(g * bpg, (g + 1) * bpg):
                nc.scalar.activation(
                    T[:, bb * Wn:(bb + 1) * Wn], io[:, :Wn], AF.Identity,
                    bias=neg_posoff[:, bb:bb + 1])
        else:
            io_b = io[:, :Wn].rearrange("p x -> p () x").broadcast_to(
                [128, bpg, Wn])
            po_b = posoff[:, bsl].rearrange("p b -> p b ()").broadcast_to(
                [128, bpg, Wn])
            nc.vector.tensor_tensor(out=T3, in0=io_b, in1=po_b,
                                    op=OP.subtract)
        nc.vector.tensor_tensor(out=Q[:, csl], in0=T[:, csl], in1=T[:, csl],
                                op=OP.mult)
        nc.scalar.activation(L[:, csl], Q[:, csl], AF.Exp, scale=-INV_S2)
    return L


def body(tc, outs, ins):
    nc = tc.nc
    out_d = outs[0]
    locs_d, vxy_d, vzi_d, scb_d = ins
    with (tc.tile_pool(name="pool", bufs=1) as pool,
          tc.tile_pool(name="rhsp", bufs=6) as rhsp,
          tc.tile_pool(name="psump", bufs=2, space="PSUM") as psump,
          tc.tile_pool(name="dram", bufs=1, space="DRAM") as dpool):
        warm = pool.tile([128, 1], F32, name="warm", tag="warm")
        nc.gpsimd.memset(warm[:], 0.0)
        nc.scalar.activation(warm[:], warm[:], AF.Exp, scale=-1.0)
        ex = extract_slots(nc, tc, pool, dpool, locs_d)
        tb_ = fetch_values(nc, pool, vxy_d, vzi_d, scb_d, ex)
        ph, pw, pd = tb_["ph"], tb_["pw"], tb_["pd"]
        vx, vy, vz, vi = tb_["vx"], tb_["vy"], tb_["vz"], tb_["vi"]

        # position offsets
        io128 = pool.tile([128, 128], F32, name="io128", tag="io128")
        nc.gpsimd.iota(io128[:], pattern=[[1, 128]], base=0,
                       channel_multiplier=0,
                       allow_small_or_imprecise_dtypes=True)
        woff = pool.tile([128, NB], F32, name="woff", tag="woff")
        nc.gpsimd.iota(woff[:], pattern=[[16, NBUCK], [0, NBPB]], base=-4,
                       channel_multiplier=0,
                       allow_small_or_imprecise_dtypes=True)
        pox = pool.tile([128, NB], F32, name="pox", tag="pox")
        nc.vector.tensor_tensor(out=pox[:], in0=ph[:], in1=vx, op=OP.add)
        poy = pool.tile([128, NB], F32, name="poy", tag="poy")
        nc.vector.tensor_tensor(out=poy[:], in0=pw[:], in1=woff[:],
                                op=OP.subtract)
        nc.vector.tensor_tensor(out=poy[:], in0=poy[:], in1=vy, op=OP.add)
        poz = pool.tile([128, NB], F32, name="poz", tag="poz")
        nc.vector.tensor_tensor(out=poz[:], in0=pd[:], in1=vz, op=OP.add)
        # 9-grid centers: 4 + offset
        pcx = pool.tile([128, NB], F32, name="pcx", tag="pcx")
        nc.vector.tensor_scalar(out=pcx[:], in0=vx, scalar1=4.0, scalar2=None,
                                op0=OP.add)
        pcy = pool.tile([128, NB], F32, name="pcy", tag="pcy")
        nc.vector.tensor_scalar(out=pcy[:], in0=vy, scalar1=4.0, scalar2=None,
                                op0=OP.add)
        pcz = pool.tile([128, NB], F32, name="pcz", tag="pcz")
        nc.vector.tensor_scalar(out=pcz[:], in0=vz, scalar1=4.0, scalar2=None,
                                op0=OP.add)

        CX = eval_profile(nc, pool, pcx, PSF, io128, "cx")
        CY = eval_profile(nc, pool, pcy, PSF, io128, "cy")
        CZ = eval_profile(nc, pool, pcz, PSF, io128, "cz")
        npox = pool.tile([128, NB], F32, name="npox", tag="npox")
        nc.vector.tensor_scalar(out=npox[:], in0=pox[:], scalar1=-1.0,
                                scalar2=None, op0=OP.mult)
        # merged per-bucket profile block: [y(3x24)=72 | z(3x64)=192 | x(3x128)=384]
        BLK = NBPB * (WJ + WK + WH)        # 648
        OY, OZ, OX = 0, NBPB * WJ, NBPB * (WJ + WK)
        T_all = pool.tile([128, NBUCK * BLK], F16, name="T_all", tag="T_all")
        Q_all = pool.tile([128, NBUCK * BLK], F16, name="Q_all", tag="Q_all")
        L_all = pool.tile([128, NBUCK * BLK], F16, name="L_all", tag="L_all")

        def ly_sl(b):
            return slice((b // 3) * BLK + OY + (b % 3) * WJ,
                         (b // 3) * BLK + OY + (b % 3) * WJ + WJ)

        def lz_sl(b):
            return slice((b // 3) * BLK + OZ + (b % 3) * WK,
                         (b // 3) * BLK + OZ + (b % 3) * WK + WK)

        def lh_sl(b):
            return slice((b // 3) * BLK + OX + (b % 3) * WH,
                         (b // 3) * BLK + OX + (b % 3) * WH + WH)

        for g in range(NBUCK):
            base = g * BLK
            bsl = slice(g * NBPB, (g + 1) * NBPB)
            for off, po, Wn in ((OY, poy, WJ), (OZ, poz, WK)):
                csl = slice(base + off, base + off + NBPB * Wn)
                io_b = io128[:, :Wn].rearrange("p x -> p () x").broadcast_to(
                    [128, NBPB, Wn])
                po_b = po[:, bsl].rearrange("p b -> p b ()").broadcast_to(
                    [128, NBPB, Wn])
                nc.vector.tensor_tensor(
                    out=T_all[:, csl].rearrange("p (b x) -> p b x", x=Wn),
                    in0=io_b, in1=po_b, op=OP.subtract)
            for bb in range(g * NBPB, (g + 1) * NBPB):
                xs = base + OX + (bb % 3) * WH
                nc.scalar.activation(
                    T_all[:, xs:xs + WH], io128[:, :WH], AF.Identity,
                    bias=npox[:, bb:bb + 1])
            blk = slice(base, base + BLK)
            nc.vector.tensor_tensor(out=Q_all[:, blk], in0=T_all[:, blk],
                                    in1=T_all[:, blk], op=OP.mult)
            nc.scalar.activation(L_all[:, blk], Q_all[:, blk], AF.Exp,
                                 scale=-INV_S2)

        # normalization: amp = vi / (sum CX * sum CY * sum CZ)
        s_x = pool.tile([128, NB], F32, name="sx", tag="sx")
        s_y = pool.tile([128, NB], F32, name="sy", tag="sy")
        s_z = pool.tile([128, NB], F32, name="sz", tag="sz")
        for s_t, C in ((s_x, CX), (s_y, CY), (s_z, CZ)):
            nc.vector.tensor_reduce(
                out=s_t[:], axis=X,
                in_=C[:].rearrange("p (b x) -> p b x", x=PSF), op=OP.add)
        nc.vector.tensor_tensor(out=s_x[:], in0=s_x[:], in1=s_y[:], op=OP.mult)
        nc.vector.tensor_tensor(out=s_x[:], in0=s_x[:], in1=s_z[:], op=OP.mult)
        nc.vector.reciprocal(s_y[:], s_x[:])
        amp = pool.tile([128, NB], F32, name="amp", tag="amp")
        nc.vector.tensor_tensor(out=amp[:], in0=s_y[:], in1=vi[:], op=OP.mult)
        LHa = pool.tile([128, NB * WH], F16, name="LHa", tag="LHa")

        out_t = pool.tile([128, W * D], F32, name="out_t", tag="out_t")
        flushed = 0
        for tb in range(NBUCK):
            ps = psump.tile([128, WJ * WK], F32, name=f"ps{tb}", tag="ps")
            for b in range(tb * NBPB, (tb + 1) * NBPB):
                nc.scalar.activation(
                    LHa[:, b * WH:(b + 1) * WH], L_all[:, lh_sl(b)],
                    AF.Identity, scale=amp[:, b:b + 1])
            for j in range(NBPB):
                b = tb * NBPB + j
                rhs = rhsp.tile([128, WJ * WK], F16, name=f"rhs{b}", tag="rhs")
                rhs3 = rhs[:].rearrange("p (j k) -> p j k", k=WK)
                ly_b = L_all[:, ly_sl(b)].rearrange(
                    "p j -> p j ()").broadcast_to([128, WJ, WK])
                lz_b = L_all[:, lz_sl(b)].rearrange(
                    "p k -> p () k").broadcast_to([128, WJ, WK])
                eng = nc.gpsimd if b in RHS_POOL_B else nc.vector
                eng.tensor_tensor(out=rhs3, in0=ly_b, in1=lz_b, op=OP.mult)
                for c0 in range(0, WJ * WK, 512):
                    c1 = min(c0 + 512, WJ * WK)
                    nc.tensor.matmul(ps[:, c0:c1], lhsT=LHa[:, bass.ts(b, WH)],
                                     rhs=rhs[:, c0:c1],
                                     start=(j == 0), stop=(j == NBPB - 1))
            # crop: fresh region copy (Act), halo region add (DVE)
            ps3 = ps[:].rearrange("p (j k) -> p j k", k=WK)
            w_new0 = 16 * tb + 4 if tb > 0 else 0
            w_new1 = min(16 * tb + 20, W)
            j_new0 = w_new0 - (16 * tb - 4)
            out_new = out_t[:, w_new0 * D:w_new1 * D].rearrange(
                "p (j k) -> p j k", k=D)
            if ACT_COPY:
                nc.scalar.activation(
                    out_new, ps3[:, j_new0:j_new0 + (w_new1 - w_new0), :],
                    AF.Copy)
            else:
                nc.vector.tensor_copy(
                    out_new, ps3[:, j_new0:j_new0 + (w_new1 - w_new0), :])
            if tb > 0:
                w_h0 = 16 * tb - 4
                out_h = out_t[:, w_h0 * D:(w_h0 + 8) * D].rearrange(
                    "p (j k) -> p j k", k=D)
                nc.vector.tensor_tensor(
                    out=out_h, in0=out_h, in1=ps3[:, 0:8, :],
                    op=OP.add)
            if INCR_FLUSH:
                # flush final columns [flushed, 16tb+12)
                f1 = 16 * tb + 12 if tb < NBUCK - 1 else W
                nc.sync.dma_start(out=out_d[:, flushed * D:f1 * D],
                                  in_=out_t[:, flushed * D:f1 * D])
                flushed = f1
        if not INCR_FLUSH:
            nc.sync.dma_start(out=out_d[:], in_=out_t[:])


def build_nc(repeats=1):
    nc = bacc.Bacc("TRN2", target_bir_lowering=False, debug=False,
                   num_devices=N_CORES, dynamic_dma_scratch_size=65536,
                   num_swdge_queues=2)
    ins = []
    for nm in IN_NAMES:
        if nm == "scb":
            shape, dt = [128, 1], F32
        elif nm == "locs":
            shape, dt = [128, 8192], F16
        else:
            shape, dt = [128, 16384], F16
        ins.append(nc.dram_tensor(nm, shape, dt, kind="ExternalInput").ap())
    out_d = nc.dram_tensor("out", [128, W * D], F32, kind="ExternalOutput").ap()
    with tile.TileContext(nc) as tc:
        for _rep in range(repeats):
            body(tc, [out_d], ins)
    nc.compile()
    return nc


class _SpmdRunner:
    def __init__(self, nc, n_cores=N_CORES, donate=True):
        import jax
        import jax.numpy as jnp
        from jax.sharding import Mesh, PartitionSpec
        from jax.experimental.shard_map import shard_map
        from concourse import bass2jax
        from concourse.bass2jax import _bass_exec_p, partition_id_tensor
        bass2jax.install_neuronx_cc_hook()
        self.jax, self.jnp = jax, jnp
        self.n_cores = n_cores
        in_names, out_names, out_avals, zero_outs = [], [], [], []
        pname = nc.partition_id_tensor.name if nc.partition_id_tensor else None
        for alloc in nc.m.functions[0].allocations:
            if not isinstance(alloc, mybir.MemoryLocationSet):
                continue
            name = alloc.memorylocations[0].name
            if alloc.kind == "ExternalInput":
                if name != pname:
                    in_names.append(name)
            elif alloc.kind == "ExternalOutput":
                shape = tuple(alloc.tensor_shape)
                dtype = mybir.dt.np(alloc.dtype)
                out_names.append(name)
                out_avals.append(jax.core.ShapedArray(shape, dtype))
                zero_outs.append(np.zeros(shape, dtype))
        self.in_names, self.out_names = in_names, out_names
        self.out_avals, self.zero_outs = out_avals, zero_outs
        n_params, n_outs = len(in_names), len(out_avals)
        all_in = in_names + out_names + ([pname] if pname else [])

        def _fn(*args):
            operands = list(args)
            if pname is not None:
                operands.append(partition_id_tensor())
            return tuple(_bass_exec_p.bind(
                *operands, out_avals=tuple(out_avals), in_names=tuple(all_in),
                out_names=tuple(out_names), lowering_input_output_aliases=(),
                sim_require_finite=True, sim_require_nnan=True, nc=nc))

        devices = jax.devices()[:n_cores]
        mesh = Mesh(np.asarray(devices), ("core",))
        specs = (PartitionSpec("core"),)
        donate_kw = (dict(donate_argnums=tuple(
            range(n_params, n_params + n_outs))) if donate else {})
        self.sharded = jax.jit(
            shard_map(_fn, mesh=mesh, in_specs=specs * (n_params + n_outs),
                      out_specs=specs * n_outs),
            keep_unused=True, **donate_kw)

    def run(self, in_maps):
        concat = [np.concatenate([np.asarray(m[n]) for m in in_maps], axis=0)
                  for n in self.in_names]
        zeros = [self.jnp.zeros((self.n_cores * z.shape[0], *z.shape[1:]),
                                z.dtype) for z in self.zero_outs]
        outs = self.sharded(*concat, *zeros)
        self.jax.block_until_ready(outs)
        return [
            {n: np.asarray(outs[i]).reshape(self.n_cores,
                                            *self.out_avals[i].shape)[c]
             for i, n in enumerate(self.out_names)}
            for c in range(self.n_cores)]


_RUNNER_CACHE = {}


def _get_runner(repeats=1, donate=True):
    key = (repeats, donate)
    if key not in _RUNNER_CACHE:
        _RUNNER_CACHE[key] = _SpmdRunner(build_nc(repeats), donate=donate)
    return _RUNNER_CACHE[key]


def _make_in_maps(locs_3d, x_os_3d, y_os_3d, z_os_3d, ints_3d, scale):
    sc = float(np.asarray(scale).reshape(-1)[0])
    scb = np.full((128, 1), 1000.0 * sc, np.float32)
    in_maps = []
    for c in range(N_CORES):
        xs = np.asarray(x_os_3d)[c, 0]
        ys = np.asarray(y_os_3d)[c, 0]
        zs = np.asarray(z_os_3d)[c, 0]
        iv = np.asarray(ints_3d)[c, 0]
        vxy = np.stack([xs.reshape(128, 128, 64), ys.reshape(128, 128, 64)],
                       axis=2).reshape(128, 16384).astype(np.float16)
        vzi = np.stack([zs.reshape(128, 128, 64), iv.reshape(128, 128, 64)],
                       axis=2).reshape(128, 16384).astype(np.float16)
        m = {"scb": scb,
             "locs": np.asarray(locs_3d)[c, 0].reshape(128, 8192).astype(
                 np.float16),
             "vxy": np.ascontiguousarray(vxy),
             "vzi": np.ascontiguousarray(vzi)}
        in_maps.append(m)
    return in_maps


def kernel(locs_3d, x_os_3d, y_os_3d, z_os_3d, ints_3d, scale):
    runner = _get_runner()
    in_maps = _make_in_maps(locs_3d, x_os_3d, y_os_3d, z_os_3d, ints_3d, scale)
    res = runner.run(in_maps)
    out = np.stack([res[c]["out"].reshape(H, W, D) for c in range(N_CORES)])
    return out[:, None].astype(np.float32)
